# revision 18
# baseline (speedup 1.0000x reference)
"""MoE routing kernel for Trainium2 (8 NeuronCores, expert-parallel).

Problem: top-2-of-8 expert MLP with squared-ReLU, d_model=1024, d_ff=1024,
N=8192 tokens. The router (softmax + top-2, ~0.2% of FLOPs) runs on host in
float64; tokens are dispatched on host (gather + sqrt(combine-weight)
scaling — relu(sqrt(w)*z)^2 == w*relu(z)^2, so the combine weight folds into
the input and the device kernel is a plain 2-layer MLP). Data path is bf16
(inputs, weights, mid, output) with f32 PSUM accumulation — PE rate is the
same as f32r (1 cy/row) but all DMA halves, which removes the startup
weight-stream stall. Load balance: experts sorted by token count are paired
big+small; each pair is served by two cores, each core taking half of the
big and half of the small expert (capacities CA/CB identical across cores so
one SPMD program fits all 8). Host scatter-adds the per-core outputs.
"""

import sys

if "/opt/trn_rl_repo" not in sys.path:
    sys.path.insert(0, "/opt/trn_rl_repo")

import numpy as np
from ml_dtypes import bfloat16

import bass_rust
import concourse.bass as bass
import concourse.tile as tile
import concourse.tile_utils as tile_utils
from concourse import mybir
from concourse.bass_utils import run_bass_kernel_spmd
from concourse.vector_clock import ScopedClock

NUM_EXPERTS = 8
TOP_K = 2
D_MODEL = 1024
D_FF = 1024
N_CORES = 8
KC = D_MODEL // 128
FT = D_FF // 128
DT = D_MODEL // 128

BF16 = mybir.dt.bfloat16
F32 = mybir.dt.float32
F32R = mybir.dt.float32r

# Cayman has 208 KiB/partition usable; the stock constant leaves 16 KiB idle.
tile_utils.max_sbuf_usage = 208 * 1024

# ---------------------------------------------------------------------------
# Compat: this container's walrus rejects instructions carrying more than one
# sem wait ("Too many sync wait commands"). Replace the TileContext final
# drain with single-wait SP nops, and post-process the module so every
# instruction carries at most one (monotonic) wait.
# ---------------------------------------------------------------------------


def _patched_drain_and_barrier(self, tick_clock, wait_clock):
    probe = self.nc.sync.nop(nofuse=True)
    wait_clock.add_sem_waits(probe.ins, ScopedClock({None: tick_clock.global_clock}))
    si = probe.ins.sync_info
    waits = list(si.on_wait) if si is not None else []
    updates = list(si.on_update) if si is not None else []
    if len(waits) > 1:
        probe.ins.sync_info = bass_rust.SyncInfo(on_wait=[waits[0]], on_update=updates)
        for w in waits[1:]:
            extra = self.nc.sync.nop(nofuse=True)
            extra.ins.sync_info = bass_rust.SyncInfo(on_wait=[w], on_update=[])
    self.nc.sync.drain()
    self.nc.all_engine_barrier()
    assert self.sems is not None
    popped = self.nc._tile_sem_poison_stack.pop()
    assert popped is self._sem_poison
    self.nc.clear_and_free_semaphores(list(self.sems.allocated().values()))
    self.nc.all_engine_barrier()


tile.TileContext._drain_and_barrier = _patched_drain_and_barrier


def split_excess_waits(nc, limit=1):
    for fn in nc.m.functions:
        for bb in fn.blocks:
            il = bb.instructions
            i = 0
            while i < len(il):
                inst = il[i]
                si = inst.sync_info
                if si is not None and len(si.on_wait) > limit:
                    waits = list(si.on_wait)
                    movable = [w for w in waits if "ge" in (w.wait_mode or "")]
                    pinned = [w for w in waits if w not in movable]
                    keep_n = max(0, limit - len(pinned))
                    if keep_n:
                        keep = pinned + movable[len(movable) - keep_n :]
                        extra = movable[: len(movable) - keep_n]
                    else:
                        keep, extra = pinned, movable
                    if not extra:
                        i += 1
                        continue
                    nops = []
                    for w in extra:
                        nop = mybir.InstNoOp(
                            name=nc.get_next_instruction_name(), ins=[], outs=[]
                        )
                        nop.engine = inst.engine
                        nop.sync_info = bass_rust.SyncInfo(on_wait=[w], on_update=[])
                        nops.append(nop)
                    inst.sync_info = bass_rust.SyncInfo(
                        on_wait=keep, on_update=list(si.on_update)
                    )
                    for j, nop in enumerate(nops):
                        il.insert(i + j, nop)
                    i += len(nops)
                i += 1


# ---------------------------------------------------------------------------
# Capacities and token blocks.
# ---------------------------------------------------------------------------


def _seg_blocks(cap, lead=0):
    """Split a segment capacity into near-equal blocks <= 512 cols, each a
    multiple of 8 — matmul free dims that are not multiples of 8 lose
    ~20-25% PE rate (measured), and bf16 slice offsets stay 16B-aligned.
    With lead>0, the first block is tiny so its x tile is one small DMA
    and the first L1 group starts as early as possible."""
    assert cap % 8 == 0 and cap >= 256
    sizes = []
    if lead and cap - lead >= 512:
        sizes.append(lead)
        cap -= lead
    nb = -(-cap // 512)
    base = cap // nb // 8 * 8
    rest = [base] * nb
    rest[-1] += cap - base * nb
    sizes += rest
    assert all(s <= 512 and s % 8 == 0 for s in sizes), sizes
    blocks, t = [], 0
    for tb in sizes:
        blocks.append((t, tb))
        t += tb
    return blocks


N_WARM = 15  # bf16 512-col warm matmuls: cover the PE clock ungate + the
# first x/w DMA window (~7.9 -> ~14.2us at gated clock).


def build_program(CA, CB):
    """One SPMD program: segment A ([0,CA), weights w1a/w2a) then segment B
    ([CA,CA+CB), weights w1b/w2b)."""
    cap = CA + CB
    blkA = _seg_blocks(CA, lead=256)
    blocks = [(t0, tb, 0) for t0, tb in blkA] + [
        (CA + t0, tb, 1) for t0, tb in _seg_blocks(CB)
    ]
    nb = len(blocks)
    nbA = len(blkA)

    nc = bass.Bass("TRN2", target_bir_lowering=False, debug=False, num_devices=N_CORES)
    # xP: host-packed [128, KC*cap]; token block (t0,tb) occupies columns
    # [KC*t0, KC*(t0+tb)) laid out [p, (kc t)]. yP likewise [p, (dt t)].
    # Weights host-prepacked so row block m holds [p, (kc c)].
    xA = nc.declare_dram_parameter("xA", [128, KC * CA], BF16, isOutput=False)
    xB = nc.declare_dram_parameter("xB", [128, KC * CB], F32R, isOutput=False)
    w1a = nc.declare_dram_parameter("w1a", [D_MODEL // 2, D_FF * 2], BF16, isOutput=False)
    w2a = nc.declare_dram_parameter("w2a", [D_FF, D_MODEL], F32R, isOutput=False)
    w1b = nc.declare_dram_parameter("w1b", [D_MODEL, D_FF], F32R, isOutput=False)
    w2b = nc.declare_dram_parameter("w2b", [D_FF, D_MODEL], F32R, isOutput=False)
    yP = nc.declare_dram_parameter("yP", [128, DT * cap], BF16, isOutput=True)

    w1a_r = w1a.rearrange("(m p) x -> m p x", p=128)
    w2a_r = w2a.rearrange("(m p) x -> m p x", p=128)
    w1b_r = w1b.rearrange("(m p) x -> m p x", p=128)
    w2b_r = w2b.rearrange("(m p) x -> m p x", p=128)

    with tile.TileContext(nc) as tc:
        with (
            tc.tile_pool(name="wpool", bufs=1) as wpool,
            tc.tile_pool(name="xpool", bufs=1) as xpool,
            tc.tile_pool(name="mpool", bufs=2) as mpool,
            tc.tile_pool(name="tpool", bufs=4) as tpool,
            tc.tile_pool(name="opool", bufs=2) as opool,
            tc.tile_pool(name="psum", bufs=4, space="PSUM") as psum_pool,
        ):
            # w1a and w2b share one buffer (tag ring of 1): w1a's last read
            # (L1 of the final A block, ~50us) precedes w2b's first read
            # (L2 of the first B block, ~100us); the tile framework inserts
            # the WAR wait on the w2b DMA automatically.
            w1a_sb = wpool.tile([128, FT * D_MODEL], BF16, tag="w_shared")
            w2a_sb = wpool.tile([128, DT * D_FF], F32R, tag="w2a")
            w1b_sb = wpool.tile([128, FT * D_MODEL], F32R, tag="w1b")
            w1_sbs = (w1a_sb, w1b_sb)
            w2_sbs = [w2a_sb, None]  # w2b tile created at step 3 (shared buf)

            warm_w = wpool.tile([128, 128], BF16, tag="warm_w")
            warm_x = wpool.tile([128, 512], BF16, tag="warm_x")
            nc.vector.memset(warm_w[:], 0.0)
            nc.vector.memset(warm_x[:], 0.0)

            def emit_w(eng, sb, dram, ms, w=1024):
                for m in ms:
                    eng.dma_start(sb[:, m * w : (m + 1) * w], dram[m])

            # x tiles + doorbells all upfront on the (otherwise idle) sync
            # engine; transfers stream in the background on its DGE ring.
            x_sb = {}

            def emit_x(bi, eng):
                t0, tb, seg = blocks[bi]
                if seg == 0:
                    xt = xpool.tile(
                        [128, KC * tb], BF16, tag="x", bufs=3, name=f"x{bi}"
                    )
                    src_ap = xA[:, KC * t0 : KC * (t0 + tb)]
                    step = 2048  # 4KiB rows in bf16
                else:
                    xt = xpool.tile(
                        [128, KC * tb], F32R, tag="x", bufs=3, name=f"x{bi}"
                    )
                    src_ap = xB[:, KC * (t0 - CA) : KC * (t0 - CA + tb)]
                    step = 1024  # 4KiB rows in f32
                x_sb[bi] = xt
                engs = [eng, nc.gpsimd]  # tail chunks off the critical ring
                for ci, c0 in enumerate(range(0, KC * tb, step)):
                    c1 = min(c0 + step, KC * tb)
                    engs[min(ci, 1)].dma_start(xt[:, c0:c1], src_ap[:, c0:c1])

            # First-needed data first on each ring. w1a is host-packed in
            # 4KiB-row chunk pairs (2KiB rows move at ~60 GB/s on the DGE;
            # 4KiB at ~260): two pairs on scalar, two on gpsimd, x0 on sync.
            # The DGE rings come up staggered (sync ~9us, scalar ~10.7,
            # gpsimd ~12) — everything the first two blocks need rides the
            # sync ring in consumption order.
            emit_x(0, nc.sync)
            emit_w(nc.sync, w1a_sb, w1a_r, [0, 1], w=2048)
            emit_w(nc.scalar, w1a_sb, w1a_r, [2], w=2048)
            emit_w(nc.sync, w1a_sb, w1a_r, [3], w=2048)
            for bi in range(1, min(3, nb)):
                emit_x(bi, nc.sync)

            # Warm-up: the PE clock is gated (~1.2 GHz) until the HAM sees a
            # few us of sustained activity. Fill the initial DMA window with
            # dependency-free bf16 matmuls on zeroed tiles.
            for i in range(N_WARM):
                wp = psum_pool.tile([128, 512], F32, tag="ps", name=f"warm{i}")
                nc.tensor.matmul(wp[:], warm_w[:], warm_x[:], start=True, stop=True)

            mids = {}

            def l1(bi, post_ft=None):
                t0, tb, seg = blocks[bi]
                w_sb = w1_sbs[seg]
                mid_sb = mpool.tile([128, FT * tb], F32R, tag="mid", name=f"mid{bi}")
                mids[bi] = mid_sb
                for ft in range(FT):
                    ps = psum_pool.tile(
                        [128, 512], F32, tag="ps", name=f"ps{bi}_{ft}"
                    )
                    for kc in range(KC):
                        nc.tensor.matmul(
                            ps[:, :tb],
                            w_sb[
                                :,
                                ft * D_MODEL + kc * 128 : ft * D_MODEL + kc * 128 + 128,
                            ],
                            x_sb[bi][:, kc * tb : (kc + 1) * tb],
                            start=(kc == 0),
                            stop=(kc == KC - 1),
                        )
                    tmp = tpool.tile([128, 512], F32, tag="tmp", name=f"tmp{bi}_{ft}")
                    nc.scalar.activation(
                        tmp[:, :tb], ps[:, :tb], mybir.ActivationFunctionType.Relu
                    )
                    nc.vector.tensor_mul(
                        mid_sb[:, ft * tb : (ft + 1) * tb], tmp[:, :tb], tmp[:, :tb]
                    )
                    if post_ft is not None:
                        post_ft(ft)

            def l2(bi):
                t0, tb, seg = blocks[bi]
                w_sb = w2_sbs[seg]
                mid_sb = mids.pop(bi)
                o_sb = opool.tile([128, DT * tb], BF16, tag="o", name=f"o{bi}")
                last = bi == nb - 1
                for dt_ in range(DT):
                    ps2 = psum_pool.tile(
                        [128, 512], F32, tag="ps2", name=f"ps2{bi}_{dt_}"
                    )
                    for fc in range(FT):
                        nc.tensor.matmul(
                            ps2[:, :tb],
                            w_sb[
                                :, dt_ * D_FF + fc * 128 : dt_ * D_FF + fc * 128 + 128
                            ],
                            mid_sb[:, fc * tb : (fc + 1) * tb],
                            start=(fc == 0),
                            stop=(fc == FT - 1),
                        )
                    nc.vector.tensor_copy(
                        o_sb[:, dt_ * tb : (dt_ + 1) * tb], ps2[:, :tb]
                    )
                    # Output DMA: halves per block mid-kernel (gpsimd ring);
                    # per-dt on sync for the last block so the tail drains in
                    # ~one 128KB transfer instead of a 1MB one.
                    if last:
                        if dt_ % 2 == 1:
                            c0 = (dt_ - 1) * tb
                            c1 = (dt_ + 1) * tb
                            eng = nc.sync if (dt_ // 2) % 2 == 0 else nc.gpsimd
                            eng.dma_start(
                                yP[:, DT * t0 + c0 : DT * t0 + c1], o_sb[:, c0:c1]
                            )
                    elif dt_ in (DT // 2 - 1, DT - 1):
                        c0 = (dt_ + 1 - DT // 2) * tb
                        c1 = (dt_ + 1) * tb
                        nc.gpsimd.dma_start(
                            yP[:, DT * t0 + c0 : DT * t0 + c1], o_sb[:, c0:c1]
                        )

            LA = 1  # mid tiles live LA+1 blocks -> mpool bufs = LA+1
            for step in range(nb + LA):
                # x ring is 3 deep: block s+2's doorbell can only be emitted
                # once block s-1's readers (its ring predecessor) exist.
                if step == 2:
                    emit_w(nc.sync, w1b_sb, w1b_r, range(8))
                if step >= 1 and step + 2 < nb:
                    emit_x(step + 2, nc.sync)
                if step < nb:
                    if step == 0:
                        # w2a (needed at ~45us) rides behind block 0's relus
                        # on the scalar ring, clearing the early window for
                        # the critical x0 + w1a stream.
                        l1(
                            0,
                            post_ft=lambda ft: emit_w(
                                nc.scalar, w2a_sb, w2a_r, [ft]
                            ),
                        )
                    else:
                        l1(step)
                if step == nbA:
                    # Second tile on the shared tag reuses w1a's buffer; the
                    # DMA waits for w1a's last reader automatically.
                    w2b_sb = wpool.tile(
                        [128, DT * D_FF], F32R, tag="w_shared", name="w2b_sb"
                    )
                    w2_sbs[1] = w2b_sb
                    emit_w(nc.gpsimd, w2b_sb, w2b_r, range(8))
                if step >= LA:
                    l2(step - LA)

    split_excess_waits(nc, limit=1)
    return nc


_PROGRAM_CACHE = {}


def _get_program(CA, CB):
    if (CA, CB) not in _PROGRAM_CACHE:
        _PROGRAM_CACHE[(CA, CB)] = build_program(CA, CB)
    return _PROGRAM_CACHE[(CA, CB)]


# ---------------------------------------------------------------------------
# Host side: routing, dispatch, combine.
# ---------------------------------------------------------------------------


def _pack_blocked(aT, cap, blocks):
    """[1024, cap] feature-major -> [128, 8*cap], each token block laid out
    [p, (g t)] so the device moves one contiguous chunk per block."""
    g = aT.shape[0] // 128
    out = np.empty((128, g * cap), aT.dtype)
    for t0, tb in blocks:
        out[:, g * t0 : g * (t0 + tb)] = (
            aT[:, t0 : t0 + tb]
            .reshape(g, 128, tb)
            .transpose(1, 0, 2)
            .reshape(128, g * tb)
        )
    return out


def _unpack_blocked(aP, cap, blocks):
    g = aP.shape[1] // cap
    out = np.empty((g * 128, cap), aP.dtype)
    for t0, tb in blocks:
        blk = aP[:, g * t0 : g * (t0 + tb)].reshape(128, g, tb)
        out[:, t0 : t0 + tb] = blk.transpose(1, 0, 2).reshape(g * 128, tb)
    return out


def _prep_weight(w):
    k, m = w.shape
    return np.ascontiguousarray(
        w.reshape(k // 128, 128, m // 128, 128).transpose(2, 1, 0, 3).reshape(m, k),
        dtype=np.float32,
    )


def _roundup8(v):
    return -(-int(v) // 8) * 8


def kernel(x, Wr, W1, W2, _trace=False):
    x = np.asarray(x)
    Wr = np.asarray(Wr)
    W1 = np.asarray(W1)
    W2 = np.asarray(W2)
    B, T, C = x.shape
    N = B * T
    xf = np.ascontiguousarray(x.reshape(N, C), dtype=np.float32)

    # Router in float64 (matches jax f32 top_k selections; verified).
    logits = xf.astype(np.float64) @ Wr.astype(np.float64)
    logits -= logits.max(axis=-1, keepdims=True)
    p = np.exp(logits)
    p /= p.sum(axis=-1, keepdims=True)
    idx = np.argsort(-p, axis=-1, kind="stable")[:, :TOP_K]  # [N, K]
    wts = np.take_along_axis(p, idx, axis=-1)  # [N, K]

    # Dispatch list sorted by expert.
    flat_e = idx.ravel()
    order = np.argsort(flat_e, kind="stable")
    tok_of_pair = np.repeat(np.arange(N), TOP_K)[order]
    w_of_pair = wts.ravel()[order]
    counts = np.bincount(flat_e, minlength=NUM_EXPERTS)
    starts = np.concatenate([[0], np.cumsum(counts)[:-1]])

    # Big+small expert pairing; each pair is served by two cores, each core
    # taking (half of big, half of small). SPMD capacities are the max halves.
    eorder = np.argsort(-counts, kind="stable")
    bigs = eorder[:4]
    smalls = eorder[7:3:-1]  # paired big i <-> small i
    CA = max(256, _roundup8(max(-(-int(counts[e]) // 2) for e in bigs)))
    CB = max(256, _roundup8(max(-(-int(counts[e]) // 2) for e in smalls)))
    cap = CA + CB
    blkA = _seg_blocks(CA, lead=256)
    blkB = _seg_blocks(CB)
    blocks_flat = blkA + [(CA + t0, tb) for t0, tb in blkB]

    w1pa = {}
    w1p = {}
    w2p = {}
    for e in map(int, bigs):
        pw = _prep_weight(W1[e]).astype(bfloat16)  # [1024 (m p), 1024]
        w1pa[e] = np.ascontiguousarray(
            pw.reshape(4, 2, 128, 1024).transpose(0, 2, 1, 3).reshape(512, 2048)
        )
    for e in map(int, smalls):
        w1p[e] = _prep_weight(W1[e])
    for e in set(map(int, eorder)):
        w2p[e] = _prep_weight(W2[e])

    sqw = np.sqrt(w_of_pair).astype(np.float32)

    def _half_slices(e, half):
        s, c = int(starts[e]), int(counts[e])
        h = -(-c // 2)
        lo, hi = (s, s + h) if half == 0 else (s + h, s + c)
        return tok_of_pair[lo:hi], sqw[lo:hi]

    in_maps = []
    core_tok = []
    for j in range(4):
        eA, eB = int(bigs[j]), int(smalls[j])
        for half in range(2):
            toksA, wA = _half_slices(eA, half)
            toksB, wB = _half_slices(eB, half)
            xTa = np.zeros((C, CA), bfloat16)
            xTa[:, : len(toksA)] = (xf[toksA] * wA[:, None]).astype(bfloat16).T
            xTb = np.zeros((C, CB), np.float32)
            xTb[:, : len(toksB)] = (xf[toksB] * wB[:, None]).T
            in_maps.append(
                {
                    "xA": _pack_blocked(xTa, CA, blkA),
                    "xB": _pack_blocked(xTb, CB, blkB),
                    "w1a": w1pa[eA],
                    "w2a": w2p[eA],
                    "w1b": w1p[eB],
                    "w2b": w2p[eB],
                }
            )
            core_tok.append((toksA, toksB))

    nc = _get_program(CA, CB)
    res = run_bass_kernel_spmd(nc, in_maps, core_ids=list(range(N_CORES)), trace=_trace)

    out = np.zeros((N, C), np.float32)
    for ci in range(N_CORES):
        toksA, toksB = core_tok[ci]
        yT = _unpack_blocked(res.results[ci]["yP"], cap, blocks_flat).astype(
            np.float32
        )
        if len(toksA):
            out[toksA] += yT[:, : len(toksA)].T
        if len(toksB):
            out[toksB] += yT[:, CA : CA + len(toksB)].T
    if _trace:
        kernel._last_exec_time_ns = res.exec_time_ns
    return out.reshape(B, T, C)


# revision 19
# speedup vs baseline: 1.0414x; 1.0414x over previous
"""MoE routing kernel for Trainium2 (8 NeuronCores, expert-parallel).

Problem: top-2-of-8 expert MLP with squared-ReLU, d_model=1024, d_ff=1024,
N=8192 tokens. The router (softmax + top-2, ~0.2% of FLOPs) runs on host in
float64; tokens are dispatched on host (gather + sqrt(combine-weight)
scaling — relu(sqrt(w)*z)^2 == w*relu(z)^2, so the combine weight folds into
the input and the device kernel is a plain 2-layer MLP). Mixed precision: segment A's first layer runs in bf16 (halved early DMA
removes the startup weight-stream stall); everything later runs f32r, which
streams at 2.4 GHz on this part (bf16-only streams at 2.0). Output is bf16. Load balance: experts sorted by token count are paired
big+small; each pair is served by two cores, each core taking half of the
big and half of the small expert (capacities CA/CB identical across cores so
one SPMD program fits all 8). Host scatter-adds the per-core outputs.
"""

import sys

if "/opt/trn_rl_repo" not in sys.path:
    sys.path.insert(0, "/opt/trn_rl_repo")

import numpy as np
from ml_dtypes import bfloat16

import bass_rust
import concourse.bass as bass
import concourse.tile as tile
import concourse.tile_utils as tile_utils
from concourse import mybir
from concourse.bass_utils import run_bass_kernel_spmd
from concourse.vector_clock import ScopedClock

NUM_EXPERTS = 8
TOP_K = 2
D_MODEL = 1024
D_FF = 1024
N_CORES = 8
KC = D_MODEL // 128
FT = D_FF // 128
DT = D_MODEL // 128

BF16 = mybir.dt.bfloat16
F32 = mybir.dt.float32
F32R = mybir.dt.float32r

# Cayman has 208 KiB/partition usable; the stock constant leaves 16 KiB idle.
tile_utils.max_sbuf_usage = 208 * 1024

# ---------------------------------------------------------------------------
# Compat: this container's walrus rejects instructions carrying more than one
# sem wait ("Too many sync wait commands"). Replace the TileContext final
# drain with single-wait SP nops, and post-process the module so every
# instruction carries at most one (monotonic) wait.
# ---------------------------------------------------------------------------


def _patched_drain_and_barrier(self, tick_clock, wait_clock):
    probe = self.nc.sync.nop(nofuse=True)
    wait_clock.add_sem_waits(probe.ins, ScopedClock({None: tick_clock.global_clock}))
    si = probe.ins.sync_info
    waits = list(si.on_wait) if si is not None else []
    updates = list(si.on_update) if si is not None else []
    if len(waits) > 1:
        probe.ins.sync_info = bass_rust.SyncInfo(on_wait=[waits[0]], on_update=updates)
        for w in waits[1:]:
            extra = self.nc.sync.nop(nofuse=True)
            extra.ins.sync_info = bass_rust.SyncInfo(on_wait=[w], on_update=[])
    self.nc.sync.drain()
    self.nc.all_engine_barrier()
    assert self.sems is not None
    popped = self.nc._tile_sem_poison_stack.pop()
    assert popped is self._sem_poison
    self.nc.clear_and_free_semaphores(list(self.sems.allocated().values()))
    self.nc.all_engine_barrier()


tile.TileContext._drain_and_barrier = _patched_drain_and_barrier


def split_excess_waits(nc, limit=1):
    for fn in nc.m.functions:
        for bb in fn.blocks:
            il = bb.instructions
            i = 0
            while i < len(il):
                inst = il[i]
                si = inst.sync_info
                if si is not None and len(si.on_wait) > limit:
                    waits = list(si.on_wait)
                    movable = [w for w in waits if "ge" in (w.wait_mode or "")]
                    pinned = [w for w in waits if w not in movable]
                    keep_n = max(0, limit - len(pinned))
                    if keep_n:
                        keep = pinned + movable[len(movable) - keep_n :]
                        extra = movable[: len(movable) - keep_n]
                    else:
                        keep, extra = pinned, movable
                    if not extra:
                        i += 1
                        continue
                    nops = []
                    for w in extra:
                        nop = mybir.InstNoOp(
                            name=nc.get_next_instruction_name(), ins=[], outs=[]
                        )
                        nop.engine = inst.engine
                        nop.sync_info = bass_rust.SyncInfo(on_wait=[w], on_update=[])
                        nops.append(nop)
                    inst.sync_info = bass_rust.SyncInfo(
                        on_wait=keep, on_update=list(si.on_update)
                    )
                    for j, nop in enumerate(nops):
                        il.insert(i + j, nop)
                    i += len(nops)
                i += 1


# ---------------------------------------------------------------------------
# Capacities and token blocks.
# ---------------------------------------------------------------------------


def _seg_blocks(cap, lead=0):
    """Split a segment capacity into near-equal blocks <= 512 cols, each a
    multiple of 8 — matmul free dims that are not multiples of 8 lose
    ~20-25% PE rate (measured), and bf16 slice offsets stay 16B-aligned.
    With lead>0, the first block is tiny so its x tile is one small DMA
    and the first L1 group starts as early as possible."""
    assert cap % 8 == 0 and cap >= 256
    sizes = []
    if lead and cap - lead >= 512:
        sizes.append(lead)
        cap -= lead
    nb = -(-cap // 512)
    base = cap // nb // 8 * 8
    rest = [base] * nb
    rest[-1] += cap - base * nb
    sizes += rest
    assert all(s <= 512 and s % 8 == 0 for s in sizes), sizes
    blocks, t = [], 0
    for tb in sizes:
        blocks.append((t, tb))
        t += tb
    return blocks


N_WARM = 11  # bf16 512-col warm matmuls: cover the PE clock ungate + the
# first x/w DMA window (~7.9 -> ~12.5us at gated clock).


def build_program(CA, CB):
    """One SPMD program: segment A ([0,CA), weights w1a/w2a) then segment B
    ([CA,CA+CB), weights w1b/w2b)."""
    cap = CA + CB
    blkA = _seg_blocks(CA, lead=256)
    blocks = [(t0, tb, 0) for t0, tb in blkA] + [
        (CA + t0, tb, 1) for t0, tb in _seg_blocks(CB)
    ]
    nb = len(blocks)
    nbA = len(blkA)

    nc = bass.Bass("TRN2", target_bir_lowering=False, debug=False, num_devices=N_CORES)
    # xP: host-packed [128, KC*cap]; token block (t0,tb) occupies columns
    # [KC*t0, KC*(t0+tb)) laid out [p, (kc t)]. yP likewise [p, (dt t)].
    # Weights host-prepacked so row block m holds [p, (kc c)].
    xA = nc.declare_dram_parameter("xA", [128, KC * CA], BF16, isOutput=False)
    xB = nc.declare_dram_parameter("xB", [128, KC * CB], F32R, isOutput=False)
    w1a = nc.declare_dram_parameter("w1a", [D_MODEL // 2, D_FF * 2], BF16, isOutput=False)
    w2a = nc.declare_dram_parameter("w2a", [D_FF, D_MODEL], F32R, isOutput=False)
    w1b = nc.declare_dram_parameter("w1b", [D_MODEL, D_FF], F32R, isOutput=False)
    w2b = nc.declare_dram_parameter("w2b", [D_FF, D_MODEL], F32R, isOutput=False)
    yP = nc.declare_dram_parameter("yP", [128, DT * cap], BF16, isOutput=True)

    w1a_r = w1a.rearrange("(m p) x -> m p x", p=128)
    w2a_r = w2a.rearrange("(m p) x -> m p x", p=128)
    w1b_r = w1b.rearrange("(m p) x -> m p x", p=128)
    w2b_r = w2b.rearrange("(m p) x -> m p x", p=128)

    with tile.TileContext(nc) as tc:
        with (
            tc.tile_pool(name="wpool", bufs=1) as wpool,
            tc.tile_pool(name="xpool", bufs=1) as xpool,
            tc.tile_pool(name="mpool", bufs=2) as mpool,
            tc.tile_pool(name="tpool", bufs=4) as tpool,
            tc.tile_pool(name="opool", bufs=2) as opool,
            tc.tile_pool(name="psum", bufs=4, space="PSUM") as psum_pool,
        ):
            # w1a and w2b share one buffer (tag ring of 1): w1a's last read
            # (L1 of the final A block, ~50us) precedes w2b's first read
            # (L2 of the first B block, ~100us); the tile framework inserts
            # the WAR wait on the w2b DMA automatically.
            w1a_sb = wpool.tile([128, FT * D_MODEL], BF16, tag="w_shared")
            w2a_sb = wpool.tile([128, DT * D_FF], F32R, tag="w2a")
            w1b_sb = wpool.tile([128, FT * D_MODEL], F32R, tag="w1b")
            w1_sbs = (w1a_sb, w1b_sb)
            w2_sbs = [w2a_sb, None]  # w2b tile created at step 3 (shared buf)

            warm_w = wpool.tile([128, 128], BF16, tag="warm_w")
            warm_x = wpool.tile([128, 512], BF16, tag="warm_x")
            nc.vector.memset(warm_w[:], 0.0)
            nc.vector.memset(warm_x[:], 0.0)

            def emit_w(eng, sb, dram, ms, w=1024):
                for m in ms:
                    eng.dma_start(sb[:, m * w : (m + 1) * w], dram[m])

            # x tiles + doorbells all upfront on the (otherwise idle) sync
            # engine; transfers stream in the background on its DGE ring.
            x_sb = {}

            def emit_x(bi, eng):
                t0, tb, seg = blocks[bi]
                if seg == 0:
                    xt = xpool.tile(
                        [128, KC * tb], BF16, tag="x", bufs=3, name=f"x{bi}"
                    )
                    src_ap = xA[:, KC * t0 : KC * (t0 + tb)]
                    step = 2048  # 4KiB rows in bf16
                else:
                    xt = xpool.tile(
                        [128, KC * tb], F32R, tag="x", bufs=3, name=f"x{bi}"
                    )
                    src_ap = xB[:, KC * (t0 - CA) : KC * (t0 - CA + tb)]
                    step = 1024  # 4KiB rows in f32
                x_sb[bi] = xt
                for c0 in range(0, KC * tb, step):
                    c1 = min(c0 + step, KC * tb)
                    eng.dma_start(xt[:, c0:c1], src_ap[:, c0:c1])

            # First-needed data first on each ring. w1a is host-packed in
            # 4KiB-row chunk pairs (2KiB rows move at ~60 GB/s on the DGE;
            # 4KiB at ~260): two pairs on scalar, two on gpsimd, x0 on sync.
            # The DGE rings come up staggered (sync ~9us, scalar ~10.7,
            # gpsimd ~12) — everything the first two blocks need rides the
            # sync ring in consumption order.
            emit_x(0, nc.sync)
            emit_w(nc.sync, w1a_sb, w1a_r, [0, 1], w=2048)
            emit_w(nc.scalar, w1a_sb, w1a_r, [2], w=2048)
            emit_w(nc.sync, w1a_sb, w1a_r, [3], w=2048)
            for bi in range(1, min(3, nb)):
                emit_x(bi, nc.sync)

            # Warm-up: the PE clock is gated (~1.2 GHz) until the HAM sees a
            # few us of sustained activity. Fill the initial DMA window with
            # dependency-free bf16 matmuls on zeroed tiles.
            for i in range(N_WARM):
                wp = psum_pool.tile([128, 512], F32, tag="ps", name=f"warm{i}")
                nc.tensor.matmul(wp[:], warm_w[:], warm_x[:], start=True, stop=True)

            mids = {}

            def l1(bi, post_ft=None):
                t0, tb, seg = blocks[bi]
                w_sb = w1_sbs[seg]
                mid_sb = mpool.tile([128, FT * tb], F32R, tag="mid", name=f"mid{bi}")
                mids[bi] = mid_sb
                for ft in range(FT):
                    ps = psum_pool.tile(
                        [128, 512], F32, tag="ps", name=f"ps{bi}_{ft}"
                    )
                    for kc in range(KC):
                        nc.tensor.matmul(
                            ps[:, :tb],
                            w_sb[
                                :,
                                ft * D_MODEL + kc * 128 : ft * D_MODEL + kc * 128 + 128,
                            ],
                            x_sb[bi][:, kc * tb : (kc + 1) * tb],
                            start=(kc == 0),
                            stop=(kc == KC - 1),
                        )
                    tmp = tpool.tile([128, 512], F32, tag="tmp", name=f"tmp{bi}_{ft}")
                    nc.scalar.activation(
                        tmp[:, :tb], ps[:, :tb], mybir.ActivationFunctionType.Relu
                    )
                    nc.vector.tensor_mul(
                        mid_sb[:, ft * tb : (ft + 1) * tb], tmp[:, :tb], tmp[:, :tb]
                    )
                    if post_ft is not None:
                        post_ft(ft)

            def l2(bi):
                t0, tb, seg = blocks[bi]
                w_sb = w2_sbs[seg]
                mid_sb = mids.pop(bi)
                o_sb = opool.tile([128, DT * tb], BF16, tag="o", name=f"o{bi}")
                last = bi == nb - 1
                for dt_ in range(DT):
                    ps2 = psum_pool.tile(
                        [128, 512], F32, tag="ps2", name=f"ps2{bi}_{dt_}"
                    )
                    for fc in range(FT):
                        nc.tensor.matmul(
                            ps2[:, :tb],
                            w_sb[
                                :, dt_ * D_FF + fc * 128 : dt_ * D_FF + fc * 128 + 128
                            ],
                            mid_sb[:, fc * tb : (fc + 1) * tb],
                            start=(fc == 0),
                            stop=(fc == FT - 1),
                        )
                    nc.vector.tensor_copy(
                        o_sb[:, dt_ * tb : (dt_ + 1) * tb], ps2[:, :tb]
                    )
                    # Output DMA: halves per block mid-kernel (gpsimd ring);
                    # per-dt on sync for the last block so the tail drains in
                    # ~one 128KB transfer instead of a 1MB one.
                    if last:
                        if dt_ % 2 == 1:
                            c0 = (dt_ - 1) * tb
                            c1 = (dt_ + 1) * tb
                            eng = nc.sync if (dt_ // 2) % 2 == 0 else nc.gpsimd
                            eng.dma_start(
                                yP[:, DT * t0 + c0 : DT * t0 + c1], o_sb[:, c0:c1]
                            )
                    elif dt_ in (DT // 2 - 1, DT - 1):
                        c0 = (dt_ + 1 - DT // 2) * tb
                        c1 = (dt_ + 1) * tb
                        nc.gpsimd.dma_start(
                            yP[:, DT * t0 + c0 : DT * t0 + c1], o_sb[:, c0:c1]
                        )

            LA = 1  # mid tiles live LA+1 blocks -> mpool bufs = LA+1
            for step in range(nb + LA):
                # x ring is 3 deep: block s+2's doorbell can only be emitted
                # once block s-1's readers (its ring predecessor) exist.
                if step == 2:
                    emit_w(nc.sync, w1b_sb, w1b_r, range(8))
                if step >= 1 and step + 2 < nb:
                    emit_x(step + 2, nc.sync)
                if step < nb:
                    if step == 0:
                        # w2a (needed at ~45us) rides behind block 0's relus
                        # on the scalar ring, clearing the early window for
                        # the critical x0 + w1a stream.
                        l1(
                            0,
                            post_ft=lambda ft: emit_w(
                                nc.scalar, w2a_sb, w2a_r, [ft]
                            ),
                        )
                    else:
                        l1(step)
                if step == nbA:
                    # Second tile on the shared tag reuses w1a's buffer; the
                    # DMA waits for w1a's last reader automatically.
                    w2b_sb = wpool.tile(
                        [128, DT * D_FF], F32R, tag="w_shared", name="w2b_sb"
                    )
                    w2_sbs[1] = w2b_sb
                    emit_w(nc.gpsimd, w2b_sb, w2b_r, range(8))
                if step >= LA:
                    l2(step - LA)

    split_excess_waits(nc, limit=1)
    return nc


_PROGRAM_CACHE = {}


def _get_program(CA, CB):
    if (CA, CB) not in _PROGRAM_CACHE:
        _PROGRAM_CACHE[(CA, CB)] = build_program(CA, CB)
    return _PROGRAM_CACHE[(CA, CB)]


# ---------------------------------------------------------------------------
# Host side: routing, dispatch, combine.
# ---------------------------------------------------------------------------


def _pack_blocked(aT, cap, blocks):
    """[1024, cap] feature-major -> [128, 8*cap], each token block laid out
    [p, (g t)] so the device moves one contiguous chunk per block."""
    g = aT.shape[0] // 128
    out = np.empty((128, g * cap), aT.dtype)
    for t0, tb in blocks:
        out[:, g * t0 : g * (t0 + tb)] = (
            aT[:, t0 : t0 + tb]
            .reshape(g, 128, tb)
            .transpose(1, 0, 2)
            .reshape(128, g * tb)
        )
    return out


def _unpack_blocked(aP, cap, blocks):
    g = aP.shape[1] // cap
    out = np.empty((g * 128, cap), aP.dtype)
    for t0, tb in blocks:
        blk = aP[:, g * t0 : g * (t0 + tb)].reshape(128, g, tb)
        out[:, t0 : t0 + tb] = blk.transpose(1, 0, 2).reshape(g * 128, tb)
    return out


def _prep_weight(w):
    k, m = w.shape
    return np.ascontiguousarray(
        w.reshape(k // 128, 128, m // 128, 128).transpose(2, 1, 0, 3).reshape(m, k),
        dtype=np.float32,
    )


def _roundup8(v):
    return -(-int(v) // 8) * 8


def kernel(x, Wr, W1, W2, _trace=False):
    x = np.asarray(x)
    Wr = np.asarray(Wr)
    W1 = np.asarray(W1)
    W2 = np.asarray(W2)
    B, T, C = x.shape
    N = B * T
    xf = np.ascontiguousarray(x.reshape(N, C), dtype=np.float32)

    # Router in float64 (matches jax f32 top_k selections; verified).
    logits = xf.astype(np.float64) @ Wr.astype(np.float64)
    logits -= logits.max(axis=-1, keepdims=True)
    p = np.exp(logits)
    p /= p.sum(axis=-1, keepdims=True)
    idx = np.argsort(-p, axis=-1, kind="stable")[:, :TOP_K]  # [N, K]
    wts = np.take_along_axis(p, idx, axis=-1)  # [N, K]

    # Dispatch list sorted by expert.
    flat_e = idx.ravel()
    order = np.argsort(flat_e, kind="stable")
    tok_of_pair = np.repeat(np.arange(N), TOP_K)[order]
    w_of_pair = wts.ravel()[order]
    counts = np.bincount(flat_e, minlength=NUM_EXPERTS)
    starts = np.concatenate([[0], np.cumsum(counts)[:-1]])

    # Big+small expert pairing; each pair is served by two cores, each core
    # taking (half of big, half of small). SPMD capacities are the max halves.
    eorder = np.argsort(-counts, kind="stable")
    bigs = eorder[:4]
    smalls = eorder[7:3:-1]  # paired big i <-> small i
    CA = max(256, _roundup8(max(-(-int(counts[e]) // 2) for e in bigs)))
    CB = max(256, _roundup8(max(-(-int(counts[e]) // 2) for e in smalls)))
    cap = CA + CB
    blkA = _seg_blocks(CA, lead=256)
    blkB = _seg_blocks(CB)
    blocks_flat = blkA + [(CA + t0, tb) for t0, tb in blkB]

    w1pa = {}
    w1p = {}
    w2p = {}
    for e in map(int, bigs):
        pw = _prep_weight(W1[e]).astype(bfloat16)  # [1024 (m p), 1024]
        w1pa[e] = np.ascontiguousarray(
            pw.reshape(4, 2, 128, 1024).transpose(0, 2, 1, 3).reshape(512, 2048)
        )
    for e in map(int, smalls):
        w1p[e] = _prep_weight(W1[e])
    for e in set(map(int, eorder)):
        w2p[e] = _prep_weight(W2[e])

    sqw = np.sqrt(w_of_pair).astype(np.float32)

    def _half_slices(e, half):
        s, c = int(starts[e]), int(counts[e])
        h = -(-c // 2)
        lo, hi = (s, s + h) if half == 0 else (s + h, s + c)
        return tok_of_pair[lo:hi], sqw[lo:hi]

    in_maps = []
    core_tok = []
    for j in range(4):
        eA, eB = int(bigs[j]), int(smalls[j])
        for half in range(2):
            toksA, wA = _half_slices(eA, half)
            toksB, wB = _half_slices(eB, half)
            xTa = np.zeros((C, CA), bfloat16)
            xTa[:, : len(toksA)] = (xf[toksA] * wA[:, None]).astype(bfloat16).T
            xTb = np.zeros((C, CB), np.float32)
            xTb[:, : len(toksB)] = (xf[toksB] * wB[:, None]).T
            in_maps.append(
                {
                    "xA": _pack_blocked(xTa, CA, blkA),
                    "xB": _pack_blocked(xTb, CB, blkB),
                    "w1a": w1pa[eA],
                    "w2a": w2p[eA],
                    "w1b": w1p[eB],
                    "w2b": w2p[eB],
                }
            )
            core_tok.append((toksA, toksB))

    nc = _get_program(CA, CB)
    res = run_bass_kernel_spmd(nc, in_maps, core_ids=list(range(N_CORES)), trace=_trace)

    out = np.zeros((N, C), np.float32)
    for ci in range(N_CORES):
        toksA, toksB = core_tok[ci]
        yT = _unpack_blocked(res.results[ci]["yP"], cap, blocks_flat).astype(
            np.float32
        )
        if len(toksA):
            out[toksA] += yT[:, : len(toksA)].T
        if len(toksB):
            out[toksB] += yT[:, CA : CA + len(toksB)].T
    if _trace:
        kernel._last_exec_time_ns = res.exec_time_ns
    return out.reshape(B, T, C)
